# revision 20
# baseline (speedup 1.0000x reference)
"""Trainium2 Bass kernel for the DiagonalSSMBlock problem.

Math (per batch, sharded one batch per core over 8 cores):
    a = -exp(log_neg_real) + i*imag ; a_bar = exp(a) = r * e^{i theta}
    b_bar = ((a_bar-1)/a)[:,None] * B
    Bu_t = b_bar @ u_t                         (complex, state dim 64)
    h_t = a_bar * h_{t-1} + Bu_t               (diagonal complex scan over L)
    y_t = Re(C @ h_t) + D*u_t ; out = LN(u + y) * gamma + beta

The rel-err budget is 2e-2, so everything runs in single-pass bf16:
  * u ships twice in bf16 (natural layout for the residual, transposed for
    the Bu matmul), both pre-tiled on host so every DMA is 128 partitions
    x 8 KiB contiguous. Output is stored bf16 and upcast on host.
  * Bu = b_bar @ u^T is 8 accumulating bf16 matmuls per 512-wide l-tile
    (K=1024). Readout y = h^T @ C^T packs two 128-row l-subtiles on
    partition halves (K=64 each), one bf16 pass.
  * The complex scan is rotated into a per-lane REAL damped scan:
    g_t = r*g_{t-1} + w_t with w_t = e^{-i theta t} Bu_t (elementwise
    rotation against bf16 cos/sin tables), h_re_t = Re(e^{i theta t} g_t).
    The rotation/scan chain is all-bf16 to hit DVE 2x mode.
  * Residual + LN: x = y + u via DVE stt (accumulates sum(x)); sum(x^2)
    via ACT Square accum; sd = Sqrt(q2*scale + eps) folds the 1/DM scale;
    normalize is split DVE(tensor_scalar)/ACT(Identity) for engine balance.
  * Bu PSUM results are copied to a bf16 SBUF stash at stage A, so PSUM
    needs only 2 banks for Bu + 6 banks (3 x [128,1024]) for y.
  * DMA: loads issue at iteration start, the store for a tile is delayed
    one iteration so it never head-of-line blocks the next tile's loads.
"""

import numpy as np

import concourse.mybir as mybir
import concourse.tile as tile
from concourse import bacc, bass_utils
from concourse.bass import MemorySpace
from concourse.mybir import ActivationFunctionType as act
from concourse.mybir import AluOpType as alu

F32 = mybir.dt.float32
BF16 = mybir.dt.bfloat16
P = 128          # partitions
L = 4096         # sequence length per core
DM = 1024        # d_model
NS = 64          # d_state
LT = 512         # l-tile (scan slice, matmul moving width)
NSUB = LT // P   # 4 l-subtiles of 128 rows per l-tile
NT = L // LT     # 8 l-tiles
KC = DM // P     # 8 contraction chunks of 128
NCORES = 8
LN_EPS = 1e-5
DH = 512         # d-model half (psum bank width)
NORM_DVE = 2     # of the 4 per-tile normalizes, how many run on DVE


def _build_program(use_ures: bool, use_gb: bool):
    """Builds the single-core Bass/Tile program (SPMD across 8 cores)."""
    nc = bacc.Bacc("TRN2", num_devices=NCORES, debug=False)

    ub_d = nc.dram_tensor("ub", [P, NT * NSUB * DM], BF16, kind="ExternalInput").ap()
    ut_d = nc.dram_tensor("ut", [P, NT * KC * LT], BF16, kind="ExternalInput").ap()
    bb_d = nc.dram_tensor("bb", [P, DM], BF16, kind="ExternalInput").ap()
    ct2_d = nc.dram_tensor("ct2", [P, DM], BF16, kind="ExternalInput").ap()
    trigc_d = nc.dram_tensor("trigc", [P, L], BF16, kind="ExternalInput").ap()
    trigs_d = nc.dram_tensor("trigs", [P, L], BF16, kind="ExternalInput").ap()
    rt_d = nc.dram_tensor("rt", [P, LT], F32, kind="ExternalInput").ap()
    ures_d = (
        nc.dram_tensor("ures", [P, NT * NSUB * DM], BF16, kind="ExternalInput").ap()
        if use_ures
        else None
    )
    if use_gb:
        gam_d = nc.dram_tensor("gam", [P, DM], F32, kind="ExternalInput").ap()
        bet_d = nc.dram_tensor("bet", [P, DM], F32, kind="ExternalInput").ap()
    out_d = nc.dram_tensor("out", [P, NT * NSUB * DM], BF16, kind="ExternalOutput").ap()

    with tile.TileContext(nc) as tc:
        with (
            tc.tile_pool(name="singles", bufs=1) as singles,
            tc.tile_pool(name="ut", bufs=3) as ut_pool,
            tc.tile_pool(name="ub", bufs=3) as ub_pool,
            tc.tile_pool(name="ur", bufs=3) as ur_pool,
            tc.tile_pool(name="bs", bufs=4) as bs_pool,
            tc.tile_pool(name="w", bufs=2) as w_pool,
            tc.tile_pool(name="g", bufs=3) as g_pool,
            tc.tile_pool(name="p", bufs=2) as p_pool,
            tc.tile_pool(name="h", bufs=3) as h_pool,
            tc.tile_pool(name="x", bufs=5) as x_pool,
            tc.tile_pool(name="sqs", bufs=2) as sq_pool,
            tc.tile_pool(name="st", bufs=3) as st_pool,
            tc.tile_pool(name="o", bufs=3) as o_pool,
            tc.tile_pool(name="pb", bufs=2, space=MemorySpace.PSUM) as psum_b,
            tc.tile_pool(name="py", bufs=3, space=MemorySpace.PSUM) as psum_y,
        ):
            bb_s = singles.tile([P, DM], BF16)
            nc.sync.dma_start(bb_s[:], bb_d)
            ct2_s = singles.tile([P, DM], BF16)
            nc.sync.dma_start(ct2_s[:], ct2_d)
            rt_s = singles.tile([P, LT], F32)
            nc.sync.dma_start(rt_s[:], rt_d)
            if use_gb:
                gam_s = singles.tile([P, DM], F32)
                nc.sync.dma_start(gam_s[:], gam_d)
                bet_s = singles.tile([P, DM], F32)
                nc.sync.dma_start(bet_s[:], bet_d)
            # trig tables are allocated here but DMA'd inside iteration 0,
            # after the first ut load (not needed until B1 of iteration 1)
            trigc = singles.tile([P, L], BF16)
            trigs = singles.tile([P, L], BF16)
            eps_s = singles.tile([P, 1], F32)
            nc.gpsimd.memset(eps_s[:], LN_EPS)

            g_prev = None
            stash_bs = {}
            stash_ut = {}
            stash_ub = {}
            stash_ur = {}
            stash_o = {}
            stash_h = {}
            for it in range(NT + 3):
                # ---- DMA issue (ring order: loads first, store delayed) ----
                if it < NT:
                    ut_t = ut_pool.tile([P, KC * LT], BF16, tag="ut")
                    nc.sync.dma_start(
                        ut_t[:], ut_d[:, it * KC * LT : (it + 1) * KC * LT]
                    )
                    stash_ut[it] = ut_t
                if it < NT:
                    # trig tables stream in per-tile chunks so tile 0's slice
                    # lands early instead of gating the pipeline for 2 MiB
                    lsl = slice(it * LT, (it + 1) * LT)
                    nc.sync.dma_start(trigc[:, lsl], trigc_d[:, lsl])
                    nc.sync.dma_start(trigs[:, lsl], trigs_d[:, lsl])
                j2 = it - 1
                if 0 <= j2 < NT:
                    ub_t = ub_pool.tile([P, NSUB * DM], BF16, tag="ub")
                    nc.sync.dma_start(
                        ub_t[:], ub_d[:, j2 * NSUB * DM : (j2 + 1) * NSUB * DM]
                    )
                    stash_ub[j2] = ub_t
                    if use_ures:
                        ur_t = ur_pool.tile([P, NSUB * DM], BF16, tag="ur")
                        nc.sync.dma_start(
                            ur_t[:], ures_d[:, j2 * NSUB * DM : (j2 + 1) * NSUB * DM]
                        )
                        stash_ur[j2] = ur_t
                j4 = it - 3
                if 0 <= j4 < NT:
                    o_prev = stash_o.pop(j4)
                    nc.sync.dma_start(
                        out_d[:, j4 * NSUB * DM : (j4 + 1) * NSUB * DM], o_prev[:]
                    )

                jt1 = it - 1
                jt2 = it - 2

                # ---- B1: pre-rotation + scan + post-rotation for tile it-1.
                # Emitted first: this chain is dependency-free at iteration
                # start, so the DVE chews it while PE fills B2's y tiles.
                #   w_re = c*b_re + s*b_im ; w_im = c*b_im - s*b_re
                if 0 <= jt1 < NT:
                    bs1 = stash_bs.pop(jt1)
                    l1 = jt1 * LT
                    cC1 = trigc[:, l1 : l1 + LT]
                    cS1 = trigs[:, l1 : l1 + LT]
                    m1 = w_pool.tile([P, LT], BF16, tag="m1")
                    nc.vector.tensor_tensor(m1[:], bs1[:], cC1, alu.mult)
                    # m2 holds the cross terms pre-swapped onto target halves
                    m2 = w_pool.tile([P, LT], BF16, tag="m2")
                    nc.vector.tensor_tensor(
                        m2[0:NS, :], bs1[NS:P, :], cS1[NS:P, :], alu.mult
                    )
                    nc.vector.tensor_tensor(
                        m2[NS:P, :], bs1[0:NS, :], cS1[0:NS, :], alu.mult
                    )
                    w = w_pool.tile([P, LT], BF16, tag="w")
                    nc.vector.tensor_tensor(
                        w[0:NS, :], m1[0:NS, :], m2[0:NS, :], alu.add
                    )
                    nc.vector.tensor_tensor(
                        w[NS:P, :], m1[NS:P, :], m2[NS:P, :], alu.subtract
                    )
                    g = g_pool.tile([P, LT], BF16, tag="g")
                    init = 0.0 if g_prev is None else g_prev[:, LT - 1 : LT]
                    nc.vector.tensor_tensor_scan(
                        g[:], rt_s[:], w[:], init, alu.mult, alu.add
                    )
                    g_prev = g
                    # post-rotation h_re = c*g_re - s*g_im, natural [NS, LT]
                    p5 = p_pool.tile([NS, LT], BF16, tag="p5")
                    nc.vector.tensor_tensor(p5[:], g[0:NS, :], cC1[0:NS, :], alu.mult)
                    p6 = p_pool.tile([NS, LT], BF16, tag="p6")
                    nc.vector.tensor_tensor(p6[:], g[NS:P, :], cS1[NS:P, :], alu.mult)
                    hre = h_pool.tile([NS, LT], BF16, tag="hre")
                    nc.vector.tensor_tensor(hre[:], p5[:], p6[:], alu.subtract)
                    stash_h[jt1] = hre

                # ---- B2: readout + residual + LN for tile it-2 ----
                if 0 <= jt2 < NT:
                    ub_t = stash_ub.pop(jt2)
                    ur_t = stash_ur.pop(jt2) if use_ures else ub_t
                    hre = stash_h.pop(jt2)
                    sx = st_pool.tile([P, NSUB], F32, tag="sx")
                    sq = st_pool.tile([P, NSUB], F32, tag="sq")
                    xs = []
                    for ls in range(NSUB):
                        yy = psum_y.tile([P, DM], F32, tag="y")
                        for dh in range(2):
                            sl = slice(dh * DH, (dh + 1) * DH)
                            nc.tensor.matmul(
                                yy[:, sl],
                                hre[:, ls * P : (ls + 1) * P],
                                ct2_s[0:NS, sl],
                                start=True,
                                stop=True,
                            )
                        x = x_pool.tile([P, DM], BF16, tag="x")
                        nc.vector.scalar_tensor_tensor(
                            x[:],
                            yy[:],
                            1.0,
                            ur_t[:, ls * DM : (ls + 1) * DM],
                            alu.mult,
                            alu.add,
                            accum_out=sx[:, ls : ls + 1],
                        )
                        sqs = sq_pool.tile([P, DM], BF16, tag="sqs")
                        nc.scalar.activation(
                            sqs[:], x[:], act.Square, accum_out=sq[:, ls : ls + 1]
                        )
                        xs.append(x)

                    # LN stats: var = (sq - sx^2/DM)/DM ; sd = sqrt(var + eps)
                    q1 = st_pool.tile([P, NSUB], F32, tag="q1")
                    nc.vector.tensor_tensor(q1[:], sx[:], sx[:], alu.mult)
                    q2 = st_pool.tile([P, NSUB], F32, tag="q2")
                    nc.vector.scalar_tensor_tensor(
                        q2[:], q1[:], -1.0 / DM, sq[:], alu.mult, alu.add
                    )
                    sd = st_pool.tile([P, NSUB], F32, tag="sd")
                    nc.scalar.activation(
                        sd[:], q2[:], act.Sqrt, bias=eps_s[:, 0:1], scale=1.0 / DM
                    )
                    rstd = st_pool.tile([P, NSUB], F32, tag="rstd")
                    nc.vector.reciprocal(rstd[:], sd[:])
                    nmr = st_pool.tile([P, NSUB], F32, tag="nmr")
                    nc.vector.scalar_tensor_tensor(
                        nmr[:], sx[:], -1.0 / DM, rstd[:], alu.mult, alu.mult
                    )

                    # normalize: o = x*rstd + (-mu*rstd), split DVE/ACT
                    o_t = o_pool.tile([P, NSUB * DM], BF16, tag="o")
                    for ls in range(NSUB):
                        osl = o_t[:, ls * DM : (ls + 1) * DM]
                        if ls < NORM_DVE:
                            nc.vector.tensor_scalar(
                                osl, xs[ls][:], rstd[:, ls : ls + 1],
                                nmr[:, ls : ls + 1], alu.mult, alu.add,
                            )
                        else:
                            nc.scalar.activation(
                                osl, xs[ls][:], act.Identity,
                                bias=nmr[:, ls : ls + 1], scale=rstd[:, ls : ls + 1],
                            )
                        if use_gb:
                            nc.vector.tensor_tensor(osl, osl, gam_s[:], alu.mult)
                            nc.vector.tensor_tensor(osl, osl, bet_s[:], alu.add)
                    stash_o[jt2] = o_t

                # ---- stage A: Bu matmul for tile `it`, stash result bf16 ----
                if it < NT:
                    ut_t = stash_ut.pop(it)
                    bu = psum_b.tile([P, LT], F32, tag="bu")
                    for k in range(KC):
                        nc.tensor.matmul(
                            bu[:],
                            bb_s[:, k * P : (k + 1) * P],
                            ut_t[:, k * LT : (k + 1) * LT],
                            start=(k == 0),
                            stop=(k == KC - 1),
                        )
                    bs = bs_pool.tile([P, LT], BF16, tag="bs")
                    nc.scalar.copy(bs[:], bu[:])
                    stash_bs[it] = bs
    nc.compile()
    return nc


try:
    import ml_dtypes

    ml_bf16 = ml_dtypes.bfloat16
except ImportError:  # pragma: no cover
    ml_bf16 = None


def _host_params(log_neg_real, imag, B_mat, C_mat):
    lnr = np.asarray(log_neg_real, np.float64)
    im = np.asarray(imag, np.float64)
    a = -np.exp(lnr) + 1j * im
    a_bar = np.exp(a)
    r = np.abs(a_bar)
    b_bar = ((a_bar - 1.0) / a)[:, None] * np.asarray(B_mat, np.float64)
    b_re = np.real(b_bar).astype(np.float32)
    b_im = np.imag(b_bar).astype(np.float32)
    # packed stationary operand for the Bu matmul: [K=d, M=128(re|im)] laid out
    # in SBUF as [128 partitions, KC*128] with chunk k at columns k*128:(k+1)*128
    bbT = np.concatenate([b_re, b_im], axis=0).T  # (DM, 128)
    bb = np.ascontiguousarray(
        bbT.reshape(KC, P, P).transpose(1, 0, 2).reshape(P, DM).astype(ml_bf16)
    )
    ct = np.asarray(C_mat, np.float32).T  # (NS, DM)
    ct2 = np.ascontiguousarray(np.concatenate([ct, ct], axis=0).astype(ml_bf16))
    t = np.arange(L, dtype=np.float64)
    ang = (im[:, None] * t[None, :]) % (2 * np.pi)
    cosT = np.cos(ang).astype(np.float32)
    sinT = np.sin(ang).astype(np.float32)
    trigc = np.ascontiguousarray(
        np.concatenate([cosT, cosT], axis=0).astype(ml_bf16)
    )
    trigs = np.ascontiguousarray(
        np.concatenate([sinT, sinT], axis=0).astype(ml_bf16)
    )
    rfull = np.concatenate([r, r]).astype(np.float32)
    rt = np.ascontiguousarray(np.broadcast_to(rfull[:, None], (P, LT)))
    return bb, ct2, trigc, trigs, rt


def _tile_nat(x):
    """[L, DM] -> [P, NT*NSUB*DM] bf16, 8KB-contiguous per partition per tile."""
    return np.ascontiguousarray(
        np.asarray(x, np.float32)
        .reshape(NT, NSUB, P, DM)
        .transpose(2, 0, 1, 3)
        .reshape(P, NT * NSUB * DM)
        .astype(ml_bf16)
    )


def _tile_trans(x):
    """[L, DM] -> transposed [P, NT*KC*LT] bf16 for the Bu matmul."""
    return np.ascontiguousarray(
        np.asarray(x, np.float32)
        .T.reshape(KC, P, NT, LT)
        .transpose(1, 2, 0, 3)
        .reshape(P, NT * KC * LT)
        .astype(ml_bf16)
    )


def _untile_out(o):
    """[P, NT*NSUB*DM] -> [L, DM] fp32."""
    return (
        np.asarray(o)
        .reshape(P, NT, NSUB, DM)
        .transpose(1, 2, 0, 3)
        .reshape(L, DM)
        .astype(np.float32)
    )


def _make_in_maps(u, log_neg_real, imag, B_mat, C_mat, D, gamma, beta):
    Dv = np.asarray(D, np.float32)
    gam = np.asarray(gamma, np.float32)
    bet = np.asarray(beta, np.float32)
    use_ures = bool(np.any(Dv != 0.0))
    use_gb = bool(np.any(gam != 1.0) or np.any(bet != 0.0))
    bb, ct2, trigc, trigs, rt = _host_params(log_neg_real, imag, B_mat, C_mat)
    shared = {
        "bb": bb, "ct2": ct2, "trigc": trigc, "trigs": trigs, "rt": rt,
    }
    if use_gb:
        shared["gam"] = np.ascontiguousarray(
            np.broadcast_to(gam[None, :], (P, DM)).astype(np.float32)
        )
        shared["bet"] = np.ascontiguousarray(
            np.broadcast_to(bet[None, :], (P, DM)).astype(np.float32)
        )
    u = np.asarray(u, np.float32)
    in_maps = []
    for b in range(NCORES):
        m = dict(shared)
        m["ub"] = _tile_nat(u[b])
        m["ut"] = _tile_trans(u[b])
        if use_ures:
            m["ures"] = _tile_nat(u[b] * (1.0 + Dv)[None, :])
        in_maps.append(m)
    return in_maps, use_ures, use_gb


_PROGRAM_CACHE = {}


def kernel(u, log_neg_real, imag, B_mat, C_mat, D, gamma, beta):
    in_maps, use_ures, use_gb = _make_in_maps(
        u, log_neg_real, imag, B_mat, C_mat, D, gamma, beta
    )
    key = (use_ures, use_gb)
    if key not in _PROGRAM_CACHE:
        _PROGRAM_CACHE[key] = _build_program(use_ures, use_gb)
    nc = _PROGRAM_CACHE[key]
    res = bass_utils.run_bass_kernel_spmd(nc, in_maps, core_ids=list(range(NCORES)))
    return np.stack([_untile_out(r["out"]) for r in res.results], axis=0)


# revision 24
# speedup vs baseline: 1.0105x; 1.0105x over previous
"""Trainium2 Bass kernel for the DiagonalSSMBlock problem.

Math (per batch, sharded one batch per core over 8 cores):
    a = -exp(log_neg_real) + i*imag ; a_bar = exp(a) = r * e^{i theta}
    b_bar = ((a_bar-1)/a)[:,None] * B
    Bu_t = b_bar @ u_t                         (complex, state dim 64)
    h_t = a_bar * h_{t-1} + Bu_t               (diagonal complex scan over L)
    y_t = Re(C @ h_t) + D*u_t ; out = LN(u + y) * gamma + beta

The rel-err budget is 2e-2, so everything runs in single-pass bf16:
  * u ships twice in bf16 (natural layout for the residual, transposed for
    the Bu matmul), both pre-tiled on host so every DMA is 128 partitions
    x 8 KiB contiguous. Output is stored bf16 and upcast on host.
  * Bu = b_bar @ u^T is 8 accumulating bf16 matmuls per 512-wide l-tile
    (K=1024). Readout y = h^T @ C^T packs two 128-row l-subtiles on
    partition halves (K=64 each), one bf16 pass.
  * The complex scan is rotated into a per-lane REAL damped scan:
    g_t = r*g_{t-1} + w_t with w_t = e^{-i theta t} Bu_t (elementwise
    rotation against bf16 cos/sin tables), h_re_t = Re(e^{i theta t} g_t).
    The rotation/scan chain is all-bf16 to hit DVE 2x mode.
  * Residual + LN: x = y + u via DVE stt (accumulates sum(x)); sum(x^2)
    via ACT Square accum; sd = Sqrt(q2*scale + eps) folds the 1/DM scale;
    normalize is split DVE(tensor_scalar)/ACT(Identity) for engine balance.
  * Bu PSUM results are copied to a bf16 SBUF stash at stage A, so PSUM
    needs only 2 banks for Bu + 6 banks (3 x [128,1024]) for y.
  * DMA: loads issue at iteration start, the store for a tile is delayed
    one iteration so it never head-of-line blocks the next tile's loads.
"""

import numpy as np

import concourse.mybir as mybir
import concourse.tile as tile
from concourse import bacc, bass_utils
from concourse.bass import MemorySpace
from concourse.mybir import ActivationFunctionType as act
from concourse.mybir import AluOpType as alu

F32 = mybir.dt.float32
BF16 = mybir.dt.bfloat16
P = 128          # partitions
L = 4096         # sequence length per core
DM = 1024        # d_model
NS = 64          # d_state
LT = 512         # l-tile (scan slice, matmul moving width)
NSUB = LT // P   # 4 l-subtiles of 128 rows per l-tile
NT = L // LT     # 8 l-tiles
KC = DM // P     # 8 contraction chunks of 128
NCORES = 8
LN_EPS = 1e-5
DH = 512         # d-model half (psum bank width)
NORM_DVE = 2     # of the 4 per-tile normalizes, how many run on DVE


def _build_program(use_ures: bool, use_gb: bool):
    """Builds the single-core Bass/Tile program (SPMD across 8 cores)."""
    nc = bacc.Bacc("TRN2", num_devices=NCORES, debug=False)

    ub_d = nc.dram_tensor("ub", [P, NT * NSUB * DM], BF16, kind="ExternalInput").ap()
    ut_d = nc.dram_tensor("ut", [P, NT * KC * LT], BF16, kind="ExternalInput").ap()
    bb_d = nc.dram_tensor("bb", [P, DM], BF16, kind="ExternalInput").ap()
    ct2_d = nc.dram_tensor("ct2", [P, DM], BF16, kind="ExternalInput").ap()
    trigc_d = nc.dram_tensor("trigc", [P, L], BF16, kind="ExternalInput").ap()
    trigs_d = nc.dram_tensor("trigs", [P, L], BF16, kind="ExternalInput").ap()
    rt_d = nc.dram_tensor("rt", [P, LT], F32, kind="ExternalInput").ap()
    ures_d = (
        nc.dram_tensor("ures", [P, NT * NSUB * DM], BF16, kind="ExternalInput").ap()
        if use_ures
        else None
    )
    if use_gb:
        gam_d = nc.dram_tensor("gam", [P, DM], F32, kind="ExternalInput").ap()
        bet_d = nc.dram_tensor("bet", [P, DM], F32, kind="ExternalInput").ap()
    out_d = nc.dram_tensor("out", [P, NT * NSUB * DM], BF16, kind="ExternalOutput").ap()

    with tile.TileContext(nc) as tc:
        with (
            tc.tile_pool(name="singles", bufs=1) as singles,
            tc.tile_pool(name="ut", bufs=3) as ut_pool,
            tc.tile_pool(name="ub", bufs=3) as ub_pool,
            tc.tile_pool(name="ur", bufs=3) as ur_pool,
            tc.tile_pool(name="bs", bufs=4) as bs_pool,
            tc.tile_pool(name="w", bufs=2) as w_pool,
            tc.tile_pool(name="g", bufs=3) as g_pool,
            tc.tile_pool(name="p", bufs=2) as p_pool,
            tc.tile_pool(name="h", bufs=3) as h_pool,
            tc.tile_pool(name="x", bufs=5) as x_pool,
            tc.tile_pool(name="sqs", bufs=2) as sq_pool,
            tc.tile_pool(name="st", bufs=3) as st_pool,
            tc.tile_pool(name="o", bufs=3) as o_pool,
            tc.tile_pool(name="pb", bufs=2, space=MemorySpace.PSUM) as psum_b,
            tc.tile_pool(name="py", bufs=3, space=MemorySpace.PSUM) as psum_y,
        ):
            bb_s = singles.tile([P, DM], BF16)
            nc.sync.dma_start(bb_s[:], bb_d)
            ct2_s = singles.tile([P, DM], BF16)
            nc.sync.dma_start(ct2_s[:], ct2_d)
            rt_s = singles.tile([P, LT], F32)
            nc.sync.dma_start(rt_s[:], rt_d)
            if use_gb:
                gam_s = singles.tile([P, DM], F32)
                nc.sync.dma_start(gam_s[:], gam_d)
                bet_s = singles.tile([P, DM], F32)
                nc.sync.dma_start(bet_s[:], bet_d)
            # trig tables are allocated here but DMA'd inside iteration 0,
            # after the first ut load (not needed until B1 of iteration 1)
            trigc = singles.tile([P, L], BF16)
            trigs = singles.tile([P, L], BF16)
            eps_s = singles.tile([P, 1], F32)
            nc.gpsimd.memset(eps_s[:], LN_EPS)

            g_prev = None
            stash_bs = {}
            stash_ut = {}
            stash_ub = {}
            stash_ur = {}
            stash_o = {}
            stash_g = {}
            for it in range(NT + 3):
                # ---- DMA issue (ring order: loads first, store delayed) ----
                if it < NT:
                    ut_t = ut_pool.tile([P, KC * LT], BF16, tag="ut")
                    nc.sync.dma_start(
                        ut_t[:], ut_d[:, it * KC * LT : (it + 1) * KC * LT]
                    )
                    stash_ut[it] = ut_t
                if it == 0:
                    nc.sync.dma_start(trigc[:], trigc_d)
                    nc.sync.dma_start(trigs[:], trigs_d)
                j2 = it - 1
                if 0 <= j2 < NT:
                    ub_t = ub_pool.tile([P, NSUB * DM], BF16, tag="ub")
                    nc.sync.dma_start(
                        ub_t[:], ub_d[:, j2 * NSUB * DM : (j2 + 1) * NSUB * DM]
                    )
                    stash_ub[j2] = ub_t
                    if use_ures:
                        ur_t = ur_pool.tile([P, NSUB * DM], BF16, tag="ur")
                        nc.sync.dma_start(
                            ur_t[:], ures_d[:, j2 * NSUB * DM : (j2 + 1) * NSUB * DM]
                        )
                        stash_ur[j2] = ur_t
                j4 = it - 3
                if 0 <= j4 < NT:
                    o_prev = stash_o.pop(j4)
                    nc.sync.dma_start(
                        out_d[:, j4 * NSUB * DM : (j4 + 1) * NSUB * DM], o_prev[:]
                    )

                jt1 = it - 1
                # ---- B2: post-rotation + readout + LN for tile it-2 ----
                jt2 = it - 2
                if 0 <= jt2 < NT:
                    ub_t = stash_ub.pop(jt2)
                    ur_t = stash_ur.pop(jt2) if use_ures else ub_t
                    g2 = stash_g.pop(jt2)
                    l2 = jt2 * LT
                    cC2 = trigc[:, l2 : l2 + LT]
                    cS2 = trigs[:, l2 : l2 + LT]
                    # h_re = c*g_re - s*g_im, natural [NS, LT] layout
                    p5 = p_pool.tile([NS, LT], BF16, tag="p5")
                    nc.vector.tensor_tensor(p5[:], g2[0:NS, :], cC2[0:NS, :], alu.mult)
                    p6 = p_pool.tile([NS, LT], BF16, tag="p6")
                    nc.vector.tensor_tensor(p6[:], g2[NS:P, :], cS2[NS:P, :], alu.mult)
                    hre = h_pool.tile([NS, LT], BF16, tag="hre")
                    nc.vector.tensor_tensor(hre[:], p5[:], p6[:], alu.subtract)

                    sx = st_pool.tile([P, NSUB], F32, tag="sx")
                    sq = st_pool.tile([P, NSUB], F32, tag="sq")
                    xs = []
                    for ls in range(NSUB):
                        yy = psum_y.tile([P, DM], F32, tag="y")
                        for dh in range(2):
                            sl = slice(dh * DH, (dh + 1) * DH)
                            nc.tensor.matmul(
                                yy[:, sl],
                                hre[:, ls * P : (ls + 1) * P],
                                ct2_s[0:NS, sl],
                                start=True,
                                stop=True,
                            )
                        x = x_pool.tile([P, DM], BF16, tag="x")
                        nc.vector.scalar_tensor_tensor(
                            x[:],
                            yy[:],
                            1.0,
                            ur_t[:, ls * DM : (ls + 1) * DM],
                            alu.mult,
                            alu.add,
                            accum_out=sx[:, ls : ls + 1],
                        )
                        sqs = sq_pool.tile([P, DM], BF16, tag="sqs")
                        nc.scalar.activation(
                            sqs[:], x[:], act.Square, accum_out=sq[:, ls : ls + 1]
                        )
                        xs.append(x)

                    # LN stats: var = (sq - sx^2/DM)/DM ; sd = sqrt(var + eps)
                    q1 = st_pool.tile([P, NSUB], F32, tag="q1")
                    nc.vector.tensor_tensor(q1[:], sx[:], sx[:], alu.mult)
                    q2 = st_pool.tile([P, NSUB], F32, tag="q2")
                    nc.vector.scalar_tensor_tensor(
                        q2[:], q1[:], -1.0 / DM, sq[:], alu.mult, alu.add
                    )
                    sd = st_pool.tile([P, NSUB], F32, tag="sd")
                    nc.scalar.activation(
                        sd[:], q2[:], act.Sqrt, bias=eps_s[:, 0:1], scale=1.0 / DM
                    )
                    rstd = st_pool.tile([P, NSUB], F32, tag="rstd")
                    nc.vector.reciprocal(rstd[:], sd[:])
                    nmr = st_pool.tile([P, NSUB], F32, tag="nmr")
                    nc.vector.scalar_tensor_tensor(
                        nmr[:], sx[:], -1.0 / DM, rstd[:], alu.mult, alu.mult
                    )

                    # normalize: o = x*rstd + (-mu*rstd), split DVE/ACT
                    o_t = o_pool.tile([P, NSUB * DM], BF16, tag="o")
                    for ls in range(NSUB):
                        osl = o_t[:, ls * DM : (ls + 1) * DM]
                        if ls < NORM_DVE:
                            nc.vector.tensor_scalar(
                                osl, xs[ls][:], rstd[:, ls : ls + 1],
                                nmr[:, ls : ls + 1], alu.mult, alu.add,
                            )
                        else:
                            nc.scalar.activation(
                                osl, xs[ls][:], act.Identity,
                                bias=nmr[:, ls : ls + 1], scale=rstd[:, ls : ls + 1],
                            )
                        if use_gb:
                            nc.vector.tensor_tensor(osl, osl, gam_s[:], alu.mult)
                            nc.vector.tensor_tensor(osl, osl, bet_s[:], alu.add)
                    stash_o[jt2] = o_t

                # ---- stage A: Bu matmul for tile `it`, stash result bf16 ----
                if it < NT:
                    ut_t = stash_ut.pop(it)
                    bu = psum_b.tile([P, LT], F32, tag="bu")
                    for k in range(KC):
                        nc.tensor.matmul(
                            bu[:],
                            bb_s[:, k * P : (k + 1) * P],
                            ut_t[:, k * LT : (k + 1) * LT],
                            start=(k == 0),
                            stop=(k == KC - 1),
                        )
                    bs = bs_pool.tile([P, LT], BF16, tag="bs")
                    nc.scalar.copy(bs[:], bu[:])
                    stash_bs[it] = bs

                # ---- B1: pre-rotation + combine + scan for tile it-1 ----
                #   w_re = c*b_re + s*b_im ; w_im = c*b_im - s*b_re
                if 0 <= jt1 < NT:
                    bs1 = stash_bs.pop(jt1)
                    l1 = jt1 * LT
                    cC1 = trigc[:, l1 : l1 + LT]
                    cS1 = trigs[:, l1 : l1 + LT]
                    m1 = w_pool.tile([P, LT], BF16, tag="m1")
                    nc.vector.tensor_tensor(m1[:], bs1[:], cC1, alu.mult)
                    # m2 holds the cross terms pre-swapped onto target halves
                    m2 = w_pool.tile([P, LT], BF16, tag="m2")
                    nc.vector.tensor_tensor(
                        m2[0:NS, :], bs1[NS:P, :], cS1[NS:P, :], alu.mult
                    )
                    nc.vector.tensor_tensor(
                        m2[NS:P, :], bs1[0:NS, :], cS1[0:NS, :], alu.mult
                    )
                    w = w_pool.tile([P, LT], BF16, tag="w")
                    nc.vector.tensor_tensor(
                        w[0:NS, :], m1[0:NS, :], m2[0:NS, :], alu.add
                    )
                    nc.vector.tensor_tensor(
                        w[NS:P, :], m1[NS:P, :], m2[NS:P, :], alu.subtract
                    )
                    g = g_pool.tile([P, LT], BF16, tag="g")
                    init = 0.0 if g_prev is None else g_prev[:, LT - 1 : LT]
                    nc.vector.tensor_tensor_scan(
                        g[:], rt_s[:], w[:], init, alu.mult, alu.add
                    )
                    g_prev = g
                    stash_g[jt1] = g
    nc.compile()
    return nc


try:
    import ml_dtypes

    ml_bf16 = ml_dtypes.bfloat16
except ImportError:  # pragma: no cover
    ml_bf16 = None


def _host_params(log_neg_real, imag, B_mat, C_mat):
    lnr = np.asarray(log_neg_real, np.float64)
    im = np.asarray(imag, np.float64)
    a = -np.exp(lnr) + 1j * im
    a_bar = np.exp(a)
    r = np.abs(a_bar)
    b_bar = ((a_bar - 1.0) / a)[:, None] * np.asarray(B_mat, np.float64)
    b_re = np.real(b_bar).astype(np.float32)
    b_im = np.imag(b_bar).astype(np.float32)
    # packed stationary operand for the Bu matmul: [K=d, M=128(re|im)] laid out
    # in SBUF as [128 partitions, KC*128] with chunk k at columns k*128:(k+1)*128
    bbT = np.concatenate([b_re, b_im], axis=0).T  # (DM, 128)
    bb = np.ascontiguousarray(
        bbT.reshape(KC, P, P).transpose(1, 0, 2).reshape(P, DM).astype(ml_bf16)
    )
    ct = np.asarray(C_mat, np.float32).T  # (NS, DM)
    ct2 = np.ascontiguousarray(np.concatenate([ct, ct], axis=0).astype(ml_bf16))
    t = np.arange(L, dtype=np.float64)
    ang = (im[:, None] * t[None, :]) % (2 * np.pi)
    cosT = np.cos(ang).astype(np.float32)
    sinT = np.sin(ang).astype(np.float32)
    trigc = np.ascontiguousarray(
        np.concatenate([cosT, cosT], axis=0).astype(ml_bf16)
    )
    trigs = np.ascontiguousarray(
        np.concatenate([sinT, sinT], axis=0).astype(ml_bf16)
    )
    rfull = np.concatenate([r, r]).astype(np.float32)
    rt = np.ascontiguousarray(np.broadcast_to(rfull[:, None], (P, LT)))
    return bb, ct2, trigc, trigs, rt


def _tile_nat(x):
    """[L, DM] -> [P, NT*NSUB*DM] bf16, 8KB-contiguous per partition per tile."""
    return np.ascontiguousarray(
        np.asarray(x, np.float32)
        .reshape(NT, NSUB, P, DM)
        .transpose(2, 0, 1, 3)
        .reshape(P, NT * NSUB * DM)
        .astype(ml_bf16)
    )


def _tile_trans(x):
    """[L, DM] -> transposed [P, NT*KC*LT] bf16 for the Bu matmul."""
    return np.ascontiguousarray(
        np.asarray(x, np.float32)
        .T.reshape(KC, P, NT, LT)
        .transpose(1, 2, 0, 3)
        .reshape(P, NT * KC * LT)
        .astype(ml_bf16)
    )


def _untile_out(o):
    """[P, NT*NSUB*DM] -> [L, DM] fp32."""
    return (
        np.asarray(o)
        .reshape(P, NT, NSUB, DM)
        .transpose(1, 2, 0, 3)
        .reshape(L, DM)
        .astype(np.float32)
    )


def _make_in_maps(u, log_neg_real, imag, B_mat, C_mat, D, gamma, beta):
    Dv = np.asarray(D, np.float32)
    gam = np.asarray(gamma, np.float32)
    bet = np.asarray(beta, np.float32)
    use_ures = bool(np.any(Dv != 0.0))
    use_gb = bool(np.any(gam != 1.0) or np.any(bet != 0.0))
    bb, ct2, trigc, trigs, rt = _host_params(log_neg_real, imag, B_mat, C_mat)
    shared = {
        "bb": bb, "ct2": ct2, "trigc": trigc, "trigs": trigs, "rt": rt,
    }
    if use_gb:
        shared["gam"] = np.ascontiguousarray(
            np.broadcast_to(gam[None, :], (P, DM)).astype(np.float32)
        )
        shared["bet"] = np.ascontiguousarray(
            np.broadcast_to(bet[None, :], (P, DM)).astype(np.float32)
        )
    u = np.asarray(u, np.float32)
    in_maps = []
    for b in range(NCORES):
        m = dict(shared)
        m["ub"] = _tile_nat(u[b])
        m["ut"] = _tile_trans(u[b])
        if use_ures:
            m["ures"] = _tile_nat(u[b] * (1.0 + Dv)[None, :])
        in_maps.append(m)
    return in_maps, use_ures, use_gb


_PROGRAM_CACHE = {}


def kernel(u, log_neg_real, imag, B_mat, C_mat, D, gamma, beta):
    in_maps, use_ures, use_gb = _make_in_maps(
        u, log_neg_real, imag, B_mat, C_mat, D, gamma, beta
    )
    key = (use_ures, use_gb)
    if key not in _PROGRAM_CACHE:
        _PROGRAM_CACHE[key] = _build_program(use_ures, use_gb)
    nc = _PROGRAM_CACHE[key]
    res = bass_utils.run_bass_kernel_spmd(nc, in_maps, core_ids=list(range(NCORES)))
    return np.stack([_untile_out(r["out"]) for r in res.results], axis=0)


# revision 35
# speedup vs baseline: 1.0358x; 1.0251x over previous
"""Trainium2 Bass kernel for the DiagonalSSMBlock problem.

Math (per batch, sharded one batch per core over 8 cores):
    a = -exp(log_neg_real) + i*imag ; a_bar = exp(a) = r * e^{i theta}
    b_bar = ((a_bar-1)/a)[:,None] * B
    Bu_t = b_bar @ u_t                         (complex, state dim 64)
    h_t = a_bar * h_{t-1} + Bu_t               (diagonal complex scan over L)
    y_t = Re(C @ h_t) + D*u_t ; out = LN(u + y) * gamma + beta

The rel-err budget is 2e-2, so everything runs in single-pass bf16:
  * u ships twice in bf16 (natural layout for the residual, transposed for
    the Bu matmul), both pre-tiled on host so every DMA is 128 partitions
    x 8 KiB contiguous. Output is stored bf16 and upcast on host.
  * Bu = b_bar @ u^T is 8 accumulating bf16 matmuls per 512-wide l-tile
    (K=1024). Readout y = h^T @ C^T packs two 128-row l-subtiles on
    partition halves (K=64 each), one bf16 pass.
  * The complex scan is rotated into a per-lane REAL damped scan:
    g_t = r*g_{t-1} + w_t with w_t = e^{-i theta t} Bu_t (elementwise
    rotation against bf16 cos/sin tables), h_re_t = Re(e^{i theta t} g_t).
    The rotation/scan chain is all-bf16 to hit DVE 2x mode.
  * Residual + LN: x = y + u via DVE stt (accumulates sum(x)); sum(x^2)
    via ACT Square accum; sd = Sqrt(q2*scale + eps) folds the 1/DM scale;
    normalize is split DVE(tensor_scalar)/ACT(Identity) for engine balance.
  * Bu PSUM results are copied to a bf16 SBUF stash at stage A, so PSUM
    needs only 2 banks for Bu + 6 banks (3 x [128,1024]) for y.
  * DMA: loads issue at iteration start, the store for a tile is delayed
    one iteration so it never head-of-line blocks the next tile's loads.
"""

import numpy as np

import concourse.mybir as mybir
import concourse.tile as tile
from concourse import bacc, bass_utils
from concourse.bass import MemorySpace
from concourse.mybir import ActivationFunctionType as act
from concourse.mybir import AluOpType as alu

F32 = mybir.dt.float32
BF16 = mybir.dt.bfloat16
P = 128          # partitions
L = 4096         # sequence length per core
DM = 1024        # d_model
NS = 64          # d_state
LT = 512         # l-tile (scan slice, matmul moving width)
NSUB = LT // P   # 4 l-subtiles of 128 rows per l-tile
NT = L // LT     # 8 l-tiles
KC = DM // P     # 8 contraction chunks of 128
NCORES = 8
LN_EPS = 1e-5
DH = 512         # d-model half (psum bank width)
NORM_DVE = 2     # of the 4 per-tile normalizes, how many run on DVE


def _build_program(use_ures: bool, use_gb: bool):
    """Builds the single-core Bass/Tile program (SPMD across 8 cores)."""
    nc = bacc.Bacc("TRN2", num_devices=NCORES, debug=False)

    ub_d = nc.dram_tensor("ub", [P, NT * NSUB * DM], BF16, kind="ExternalInput").ap()
    ut_d = nc.dram_tensor("ut", [P, NT * KC * LT], BF16, kind="ExternalInput").ap()
    bb_d = nc.dram_tensor("bb", [P, DM], BF16, kind="ExternalInput").ap()
    ct2_d = nc.dram_tensor("ct2", [P, DM], BF16, kind="ExternalInput").ap()
    trigc_d = nc.dram_tensor("trigc", [P, L], BF16, kind="ExternalInput").ap()
    trigs_d = nc.dram_tensor("trigs", [P, L], BF16, kind="ExternalInput").ap()
    rt_d = nc.dram_tensor("rt", [P, LT], F32, kind="ExternalInput").ap()
    ures_d = (
        nc.dram_tensor("ures", [P, NT * NSUB * DM], BF16, kind="ExternalInput").ap()
        if use_ures
        else None
    )
    if use_gb:
        gam_d = nc.dram_tensor("gam", [P, DM], F32, kind="ExternalInput").ap()
        bet_d = nc.dram_tensor("bet", [P, DM], F32, kind="ExternalInput").ap()
    out_d = nc.dram_tensor("out", [P, NT * NSUB * DM], BF16, kind="ExternalOutput").ap()

    with tile.TileContext(nc) as tc:
        with (
            tc.tile_pool(name="singles", bufs=1) as singles,
            tc.tile_pool(name="ut", bufs=3) as ut_pool,
            tc.tile_pool(name="ub", bufs=3) as ub_pool,
            tc.tile_pool(name="ur", bufs=3) as ur_pool,
            tc.tile_pool(name="bs", bufs=4) as bs_pool,
            tc.tile_pool(name="w", bufs=2) as w_pool,
            tc.tile_pool(name="g", bufs=3) as g_pool,
            tc.tile_pool(name="p", bufs=2) as p_pool,
            tc.tile_pool(name="h", bufs=3) as h_pool,
            tc.tile_pool(name="x", bufs=5) as x_pool,
            tc.tile_pool(name="sqs", bufs=2) as sq_pool,
            tc.tile_pool(name="st", bufs=3) as st_pool,
            tc.tile_pool(name="o", bufs=3) as o_pool,
            tc.tile_pool(name="pb", bufs=2, space=MemorySpace.PSUM) as psum_b,
            tc.tile_pool(name="py", bufs=3, space=MemorySpace.PSUM) as psum_y,
        ):
            bb_s = singles.tile([P, DM], BF16)
            nc.sync.dma_start(bb_s[:], bb_d)
            ct2_s = singles.tile([P, DM], BF16)
            rt_s = singles.tile([P, LT], F32)
            if use_gb:
                gam_s = singles.tile([P, DM], F32)
                nc.sync.dma_start(gam_s[:], gam_d)
                bet_s = singles.tile([P, DM], F32)
                nc.sync.dma_start(bet_s[:], bet_d)
            # trig tables are allocated here but DMA'd inside iteration 0,
            # after the first ut load (not needed until B1 of iteration 1)
            trigc = singles.tile([P, L], BF16)
            trigs = singles.tile([P, L], BF16)
            eps_s = singles.tile([P, 1], F32)
            nc.gpsimd.memset(eps_s[:], LN_EPS)

            g_prev = None
            stash_bs = {}
            stash_ut = {}
            stash_ub = {}
            stash_ur = {}
            stash_o = {}
            stash_g = {}
            for it in range(NT + 3):
                # ---- DMA issue (ring order: loads first, store delayed) ----
                if it < NT:
                    ut_t = ut_pool.tile([P, KC * LT], BF16, tag="ut")
                    nc.sync.dma_start(
                        ut_t[:], ut_d[:, it * KC * LT : (it + 1) * KC * LT]
                    )
                    stash_ut[it] = ut_t
                # trig tables stream in halves behind the first ut load, so
                # tile 0's rotation isn't gated by 2 MiB of table DMA; ct2/rt
                # (first needed at iters 1-2) follow the critical-path loads
                lh = L // 2
                if it == 0:
                    nc.sync.dma_start(trigc[:, 0:lh], trigc_d[:, 0:lh])
                    nc.sync.dma_start(trigs[:, 0:lh], trigs_d[:, 0:lh])
                    nc.sync.dma_start(rt_s[:], rt_d)
                    nc.sync.dma_start(ct2_s[:], ct2_d)
                if it == 1:
                    nc.sync.dma_start(trigc[:, lh:L], trigc_d[:, lh:L])
                    nc.sync.dma_start(trigs[:, lh:L], trigs_d[:, lh:L])
                j2 = it - 1
                if 0 <= j2 < NT:
                    ub_t = ub_pool.tile([P, NSUB * DM], BF16, tag="ub")
                    nc.sync.dma_start(
                        ub_t[:], ub_d[:, j2 * NSUB * DM : (j2 + 1) * NSUB * DM]
                    )
                    stash_ub[j2] = ub_t
                    if use_ures:
                        ur_t = ur_pool.tile([P, NSUB * DM], BF16, tag="ur")
                        nc.sync.dma_start(
                            ur_t[:], ures_d[:, j2 * NSUB * DM : (j2 + 1) * NSUB * DM]
                        )
                        stash_ur[j2] = ur_t
                j4 = it - 3
                if 0 <= j4 < NT:
                    o_prev = stash_o.pop(j4)
                    nc.sync.dma_start(
                        out_d[:, j4 * NSUB * DM : (j4 + 1) * NSUB * DM], o_prev[:]
                    )

                jt1 = it - 1
                # ---- B2: post-rotation + readout + LN for tile it-2 ----
                jt2 = it - 2
                if 0 <= jt2 < NT:
                    ub_t = stash_ub.pop(jt2)
                    ur_t = stash_ur.pop(jt2) if use_ures else ub_t
                    g2 = stash_g.pop(jt2)
                    l2 = jt2 * LT
                    cC2 = trigc[:, l2 : l2 + LT]
                    cS2 = trigs[:, l2 : l2 + LT]
                    # h_re = c*g_re - s*g_im, natural [NS, LT] layout
                    p5 = p_pool.tile([NS, LT], BF16, tag="p5")
                    nc.vector.tensor_tensor(p5[:], g2[0:NS, :], cC2[0:NS, :], alu.mult)
                    p6 = p_pool.tile([NS, LT], BF16, tag="p6")
                    nc.vector.tensor_tensor(p6[:], g2[NS:P, :], cS2[NS:P, :], alu.mult)
                    hre = h_pool.tile([NS, LT], BF16, tag="hre")
                    nc.vector.tensor_tensor(hre[:], p5[:], p6[:], alu.subtract)

                    sx = st_pool.tile([P, NSUB], F32, tag="sx")
                    sq = st_pool.tile([P, NSUB], F32, tag="sq")
                    xs = []
                    for ls in range(NSUB):
                        yy = psum_y.tile([P, DM], F32, tag="y")
                        for dh in range(2):
                            sl = slice(dh * DH, (dh + 1) * DH)
                            nc.tensor.matmul(
                                yy[:, sl],
                                hre[:, ls * P : (ls + 1) * P],
                                ct2_s[0:NS, sl],
                                start=True,
                                stop=True,
                            )
                        x = x_pool.tile([P, DM], BF16, tag="x")
                        nc.vector.scalar_tensor_tensor(
                            x[:],
                            yy[:],
                            1.0,
                            ur_t[:, ls * DM : (ls + 1) * DM],
                            alu.mult,
                            alu.add,
                            accum_out=sx[:, ls : ls + 1],
                        )
                        sqs = sq_pool.tile([P, DM], BF16, tag="sqs")
                        nc.scalar.activation(
                            sqs[:], x[:], act.Square, accum_out=sq[:, ls : ls + 1]
                        )
                        xs.append(x)

                    # LN stats: var = (sq - sx^2/DM)/DM ; sd = sqrt(var + eps)
                    q1 = st_pool.tile([P, NSUB], F32, tag="q1")
                    nc.vector.tensor_tensor(q1[:], sx[:], sx[:], alu.mult)
                    q2 = st_pool.tile([P, NSUB], F32, tag="q2")
                    nc.vector.scalar_tensor_tensor(
                        q2[:], q1[:], -1.0 / DM, sq[:], alu.mult, alu.add
                    )
                    sd = st_pool.tile([P, NSUB], F32, tag="sd")
                    nc.scalar.activation(
                        sd[:], q2[:], act.Sqrt, bias=eps_s[:, 0:1], scale=1.0 / DM
                    )
                    rstd = st_pool.tile([P, NSUB], F32, tag="rstd")
                    nc.vector.reciprocal(rstd[:], sd[:])
                    nmr = st_pool.tile([P, NSUB], F32, tag="nmr")
                    nc.vector.scalar_tensor_tensor(
                        nmr[:], sx[:], -1.0 / DM, rstd[:], alu.mult, alu.mult
                    )

                    # normalize: o = x*rstd + (-mu*rstd), split DVE/ACT
                    o_t = o_pool.tile([P, NSUB * DM], BF16, tag="o")
                    for ls in range(NSUB):
                        osl = o_t[:, ls * DM : (ls + 1) * DM]
                        if ls < NORM_DVE:
                            nc.vector.tensor_scalar(
                                osl, xs[ls][:], rstd[:, ls : ls + 1],
                                nmr[:, ls : ls + 1], alu.mult, alu.add,
                            )
                        else:
                            nc.scalar.activation(
                                osl, xs[ls][:], act.Identity,
                                bias=nmr[:, ls : ls + 1], scale=rstd[:, ls : ls + 1],
                            )
                        if use_gb:
                            nc.vector.tensor_tensor(osl, osl, gam_s[:], alu.mult)
                            nc.vector.tensor_tensor(osl, osl, bet_s[:], alu.add)
                    stash_o[jt2] = o_t

                # ---- stage A: Bu matmul for tile `it`, stash result bf16 ----
                if it < NT:
                    ut_t = stash_ut.pop(it)
                    bu = psum_b.tile([P, LT], F32, tag="bu")
                    for k in range(KC):
                        nc.tensor.matmul(
                            bu[:],
                            bb_s[:, k * P : (k + 1) * P],
                            ut_t[:, k * LT : (k + 1) * LT],
                            start=(k == 0),
                            stop=(k == KC - 1),
                        )
                    bs = bs_pool.tile([P, LT], BF16, tag="bs")
                    nc.scalar.copy(bs[:], bu[:])
                    stash_bs[it] = bs

                # ---- B1: pre-rotation + combine + scan for tile it-1 ----
                #   w_re = c*b_re + s*b_im ; w_im = c*b_im - s*b_re
                if 0 <= jt1 < NT:
                    bs1 = stash_bs.pop(jt1)
                    l1 = jt1 * LT
                    cC1 = trigc[:, l1 : l1 + LT]
                    cS1 = trigs[:, l1 : l1 + LT]
                    m1 = w_pool.tile([P, LT], BF16, tag="m1")
                    nc.vector.tensor_tensor(m1[:], bs1[:], cC1, alu.mult)
                    # m2 holds the cross terms pre-swapped onto target halves
                    m2 = w_pool.tile([P, LT], BF16, tag="m2")
                    nc.vector.tensor_tensor(
                        m2[0:NS, :], bs1[NS:P, :], cS1[NS:P, :], alu.mult
                    )
                    nc.vector.tensor_tensor(
                        m2[NS:P, :], bs1[0:NS, :], cS1[0:NS, :], alu.mult
                    )
                    w = w_pool.tile([P, LT], BF16, tag="w")
                    nc.vector.tensor_tensor(
                        w[0:NS, :], m1[0:NS, :], m2[0:NS, :], alu.add
                    )
                    nc.vector.tensor_tensor(
                        w[NS:P, :], m1[NS:P, :], m2[NS:P, :], alu.subtract
                    )
                    g = g_pool.tile([P, LT], BF16, tag="g")
                    init = 0.0 if g_prev is None else g_prev[:, LT - 1 : LT]
                    nc.vector.tensor_tensor_scan(
                        g[:], rt_s[:], w[:], init, alu.mult, alu.add
                    )
                    g_prev = g
                    stash_g[jt1] = g
    nc.compile()
    return nc


try:
    import ml_dtypes

    ml_bf16 = ml_dtypes.bfloat16
except ImportError:  # pragma: no cover
    ml_bf16 = None


def _host_params(log_neg_real, imag, B_mat, C_mat):
    lnr = np.asarray(log_neg_real, np.float64)
    im = np.asarray(imag, np.float64)
    a = -np.exp(lnr) + 1j * im
    a_bar = np.exp(a)
    r = np.abs(a_bar)
    b_bar = ((a_bar - 1.0) / a)[:, None] * np.asarray(B_mat, np.float64)
    b_re = np.real(b_bar).astype(np.float32)
    b_im = np.imag(b_bar).astype(np.float32)
    # packed stationary operand for the Bu matmul: [K=d, M=128(re|im)] laid out
    # in SBUF as [128 partitions, KC*128] with chunk k at columns k*128:(k+1)*128
    bbT = np.concatenate([b_re, b_im], axis=0).T  # (DM, 128)
    bb = np.ascontiguousarray(
        bbT.reshape(KC, P, P).transpose(1, 0, 2).reshape(P, DM).astype(ml_bf16)
    )
    ct = np.asarray(C_mat, np.float32).T  # (NS, DM)
    ct2 = np.ascontiguousarray(np.concatenate([ct, ct], axis=0).astype(ml_bf16))
    t = np.arange(L, dtype=np.float64)
    ang = (im[:, None] * t[None, :]) % (2 * np.pi)
    cosT = np.cos(ang).astype(np.float32)
    sinT = np.sin(ang).astype(np.float32)
    trigc = np.ascontiguousarray(
        np.concatenate([cosT, cosT], axis=0).astype(ml_bf16)
    )
    trigs = np.ascontiguousarray(
        np.concatenate([sinT, sinT], axis=0).astype(ml_bf16)
    )
    rfull = np.concatenate([r, r]).astype(np.float32)
    rt = np.ascontiguousarray(np.broadcast_to(rfull[:, None], (P, LT)))
    return bb, ct2, trigc, trigs, rt


def _tile_nat(x):
    """[L, DM] -> [P, NT*NSUB*DM] bf16, 8KB-contiguous per partition per tile."""
    return np.ascontiguousarray(
        np.asarray(x, np.float32)
        .reshape(NT, NSUB, P, DM)
        .transpose(2, 0, 1, 3)
        .reshape(P, NT * NSUB * DM)
        .astype(ml_bf16)
    )


def _tile_trans(x):
    """[L, DM] -> transposed [P, NT*KC*LT] bf16 for the Bu matmul."""
    return np.ascontiguousarray(
        np.asarray(x, np.float32)
        .T.reshape(KC, P, NT, LT)
        .transpose(1, 2, 0, 3)
        .reshape(P, NT * KC * LT)
        .astype(ml_bf16)
    )


def _untile_out(o):
    """[P, NT*NSUB*DM] -> [L, DM] fp32."""
    return (
        np.asarray(o)
        .reshape(P, NT, NSUB, DM)
        .transpose(1, 2, 0, 3)
        .reshape(L, DM)
        .astype(np.float32)
    )


def _make_in_maps(u, log_neg_real, imag, B_mat, C_mat, D, gamma, beta):
    Dv = np.asarray(D, np.float32)
    gam = np.asarray(gamma, np.float32)
    bet = np.asarray(beta, np.float32)
    use_ures = bool(np.any(Dv != 0.0))
    use_gb = bool(np.any(gam != 1.0) or np.any(bet != 0.0))
    bb, ct2, trigc, trigs, rt = _host_params(log_neg_real, imag, B_mat, C_mat)
    shared = {
        "bb": bb, "ct2": ct2, "trigc": trigc, "trigs": trigs, "rt": rt,
    }
    if use_gb:
        shared["gam"] = np.ascontiguousarray(
            np.broadcast_to(gam[None, :], (P, DM)).astype(np.float32)
        )
        shared["bet"] = np.ascontiguousarray(
            np.broadcast_to(bet[None, :], (P, DM)).astype(np.float32)
        )
    u = np.asarray(u, np.float32)
    in_maps = []
    for b in range(NCORES):
        m = dict(shared)
        m["ub"] = _tile_nat(u[b])
        m["ut"] = _tile_trans(u[b])
        if use_ures:
            m["ures"] = _tile_nat(u[b] * (1.0 + Dv)[None, :])
        in_maps.append(m)
    return in_maps, use_ures, use_gb


_PROGRAM_CACHE = {}


def kernel(u, log_neg_real, imag, B_mat, C_mat, D, gamma, beta):
    in_maps, use_ures, use_gb = _make_in_maps(
        u, log_neg_real, imag, B_mat, C_mat, D, gamma, beta
    )
    key = (use_ures, use_gb)
    if key not in _PROGRAM_CACHE:
        _PROGRAM_CACHE[key] = _build_program(use_ures, use_gb)
    nc = _PROGRAM_CACHE[key]
    res = bass_utils.run_bass_kernel_spmd(nc, in_maps, core_ids=list(range(NCORES)))
    return np.stack([_untile_out(r["out"]) for r in res.results], axis=0)
